# revision 20
# baseline (speedup 1.0000x reference)
"""Trainium2 Bass kernel for nn_CrossmodalFusion (B=1024, R=36, D=1024).

Data-parallel over batch across 8 NeuronCores with token-level sparsity:
tokens with j >= region_lens[b] produce exactly relu(f1_b), so the host
compacts each core's valid tokens (~51%), the device processes only those,
and the host scatters results back (masked rows filled with host-computed
relu(f1_b), which is exact).

Device layout is feature-major (features on SBUF partitions, tokens on the
free dim); every GEMM uses small replicated weights as the stationary lhsT.
The four big GEMMs (MLP1, MLP2, sc_W, f1_W) run in fp8 e4m3 with
perf_mode=DoubleRow (2 K-chunks per instruction, ~1.5x PE throughput);
weights are pre-scaled by 16 (power of two) on the host and compensated
exactly in the PSUM evictions to keep small weights out of the fp8
subnormal range. The residual path stays bf16: the host ships both
x_fp8 (for MLP input) and bf16(x + mi_b2) (residual, folded bias).

Attention is a block-diagonal bf16 matmul + indicator-mask + a single
fused ones-matmul that simultaneously reduces over the batch rows and
broadcasts the per-token score to all 128 partitions (no DRAM bounce).
seg_rep is folded into the last GEMM via qw = 16*(q @ f1_W + f1_b) rows
gathered with the indicator matrix.

Inputs/outputs stream as per-tile packed blobs (contiguous 4-8KB per
partition row); outputs return in bf16 and are upcast on the host.
DMA traffic is spread over the three issue queues (SP / Activation /
Pool) so weight, x-tile and unet transfers flow in parallel and the PE
starts within a few microseconds.

Because each core's token plan differs, 8 per-core programs are compiled
(concurrently) and dispatched asynchronously, one per NeuronCore.
"""
import os
import sys
import types
from concurrent.futures import ThreadPoolExecutor
from contextlib import ExitStack

sys.path.insert(0, "/opt/trn_rl_repo")

import numpy as np
import ml_dtypes

import concourse.bass as bass
import concourse.tile as tile
from concourse import bacc, mybir
from concourse.masks import make_identity

F32 = mybir.dt.float32
BF16 = mybir.dt.bfloat16
F8 = mybir.dt.float8e4

NPBF16 = ml_dtypes.bfloat16
NPF8 = ml_dtypes.float8_e4m3

B, R, D = 1024, 36, 1024
H = D // 2
SEG_C = 133
NCORES = 8
BC = B // NCORES            # batches per core
KC = D // 128               # 8 feature chunks
KH = H // 128               # 4 hidden chunks

TOKCAP = 512                # tokens per tile (PSUM fp32 bank width)
WSC = 16.0                  # fp8 weight pre-scale (power of 2, exact)

LAST_EXEC_NS = None
_LAST_TRACE = None

DR = mybir.MatmulPerfMode.DoubleRow


def _wire_ntff_hook():
    if "antenv.axon_hooks" in sys.modules:
        return
    try:
        import trn_agent_boot.trn_boot as tb
        hook = tb._ntff_profile_via_ctypes("/opt/axon/libaxon_pjrt.so")
    except Exception:
        hook = None
    mod = types.ModuleType("antenv.axon_hooks")
    _h = [hook]
    mod.set_axon_ntff_profile_hook = lambda h: _h.__setitem__(0, h)
    mod.get_axon_ntff_profile_hook = lambda: _h[0]
    sys.modules["antenv.axon_hooks"] = mod


def _pad16(n):
    return (n + 15) // 16 * 16


def _make_plan(lens_c):
    """Tile plan for one core from its per-batch valid-token counts."""
    stream = []  # (local_batch, j)
    for lb, ln in enumerate(lens_c):
        stream.extend((lb, j) for j in range(int(ln)))
    ntokc = len(stream)
    tiles = []
    t0 = 0
    while t0 < ntokc:
        nt = 0
        b_first = stream[t0][0]
        while t0 + nt < ntokc and nt < TOKCAP:
            lb = stream[t0 + nt][0]
            if lb - b_first + 1 > 128:
                break
            nt += 1
        b_last = stream[t0 + nt - 1][0]
        segs = []
        pos = 0
        while pos < nt:
            lb = stream[t0 + pos][0]
            end = pos
            while end < nt and stream[t0 + end][0] == lb:
                end += 1
            segs.append((lb - b_first, pos, end))
            pos = end
        tiles.append(dict(t0=t0, nt=nt, b0=b_first, nb=b_last - b_first + 1, segs=segs))
        t0 += nt
    return tiles, ntokc


def _emit(ctx, tc, plan, has_b1, has_scb):
    nc = tc.nc
    AF = mybir.ActivationFunctionType
    ALU = mybir.AluOpType
    tiles, ntokc = plan
    ntiles = len(tiles)

    # ---- DRAM I/O -------------------------------------------------------
    x8blob = nc.dram_tensor("x8blob", [ntiles, 128, KC * TOKCAP], F8,
                            kind="ExternalInput").ap()
    xbblob = nc.dram_tensor("xbblob", [ntiles, 128, KC * TOKCAP], BF16,
                            kind="ExternalInput").ap()
    unet = nc.dram_tensor("unet", [BC, SEG_C, 49], BF16, kind="ExternalInput").ap()
    ind_sz = sum(t["nb"] * _pad16(t["nt"]) for t in tiles)
    ind_blob = nc.dram_tensor("ind", [ind_sz], BF16, kind="ExternalInput").ap()
    wi = {}
    for name, shape, dt in [
        ("mi_W1", [128, KC * H], F8), ("mi_b1", [1, H], BF16),
        ("mi_W2", [128, KH * D], F8),
        ("ms_W1", [128, KC * H], BF16), ("ms_b1", [1, H], BF16),
        ("ms_W2", [128, KH * D], BF16), ("ms_b2", [1, D], BF16),
        ("seg_W", [SEG_C, D], BF16), ("seg_b", [1, D], BF16),
        ("ln_g", [1, D], BF16), ("ln_b", [1, D], BF16),
        ("sc_W", [128, KC * D], F8), ("sc_b", [1, D], BF16),
        ("f1_W", [128, KC * D], F8), ("f1_Wb", [2, 128, KC * 512], BF16),
        ("f1_b", [1, D], BF16),
    ]:
        wi[name] = nc.dram_tensor(name, shape, dt, kind="ExternalInput").ap()
    outblob = nc.dram_tensor("outblob", [ntiles, 128, KC * TOKCAP], BF16,
                             kind="ExternalOutput").ap()
    qw_scr = nc.dram_tensor("qw_scr", [BC, D], BF16).ap()

    # ---- persistent constants ------------------------------------------
    const = ctx.enter_context(tc.tile_pool(name="const", bufs=1))

    def load_w(name, kchunks, m, dt, eng):
        # host pre-packs to [128, kc*m]: per-partition rows fully contiguous
        t = const.tile([128, kchunks, m], dt, tag=f"cw_{name}")
        eng.dma_start(t[:], wi[name].rearrange("p (kc m) -> p kc m", kc=kchunks))
        return t

    # main-loop fp8 weights go on the SP (sync) queue, issued between the
    # tile-0 x loads and tile-1 loads, keeping the scalar queue compute-only
    wtiles = {}

    def load_main_weights():
        wtiles["W_mi1"] = load_w("mi_W1", KC, H, F8, nc.sync)
        wtiles["W_mi2"] = load_w("mi_W2", KH, D, F8, nc.sync)
        wtiles["W_sc"] = load_w("sc_W", KC, D, F8, nc.sync)
        wtiles["W_f1"] = load_w("f1_W", KC, D, F8, nc.sync)
        b1 = const.tile([1, H], BF16, tag="b1row")
        nc.sync.dma_start(b1[:], wi["mi_b1"])
        wtiles["b1_row"] = b1
        scb = const.tile([1, D], BF16, tag="scbrow")
        nc.sync.dma_start(scb[:], wi["sc_b"])
        wtiles["scb_row"] = scb

    ones_row = const.tile([1, 512], BF16)
    nc.vector.memset(ones_row[:], 1.0)
    ones_sq = const.tile([128, 128], BF16)
    nc.vector.memset(ones_sq[:], 1.0)
    ident_bf = const.tile([128, 128], BF16)
    make_identity(nc, ident_bf)
    eps_t = const.tile([128, 1], F32)
    nc.vector.memset(eps_t[:], 1e-5)

    qT_bf = const.tile([128, KC, BC], BF16)    # feature-major q (lhsT for attn)

    psum = ctx.enter_context(tc.tile_pool(name="psum", bufs=1, space="PSUM"))

    # ---- main-loop pools ------------------------------------------------
    xp8 = ctx.enter_context(tc.tile_pool(name="xp8", bufs=2))
    xpb = ctx.enter_context(tc.tile_pool(name="xpb", bufs=2))
    hp = ctx.enter_context(tc.tile_pool(name="hp", bufs=2))
    rp = ctx.enter_context(tc.tile_pool(name="rp", bufs=2))
    wcp = ctx.enter_context(tc.tile_pool(name="wcp", bufs=2))
    scp = ctx.enter_context(tc.tile_pool(name="scp", bufs=2))
    zp = ctx.enter_context(tc.tile_pool(name="zp", bufs=2))
    op = ctx.enter_context(tc.tile_pool(name="op", bufs=2))
    sp = ctx.enter_context(tc.tile_pool(name="sp", bufs=2))

    ind_offs = []
    off = 0
    for t in tiles:
        ind_offs.append(off)
        off += t["nb"] * _pad16(t["nt"])

    def stage1(ti, first=False):
        """x loads + MLP (mm1 fp8 DR, mm2 fp8 DR + bf16 residual) -> r_bf."""
        tl = tiles[ti]
        nt = _pad16(tl["nt"])
        x8 = xp8.tile([128, KC, nt], F8, tag="x8")
        nc.sync.dma_start(x8[:], x8blob[ti, :, 0:KC * nt].rearrange(
            "p (kc t) -> p kc t", kc=KC))
        xb = xpb.tile([128, KC, nt], BF16, tag="xb")
        nc.sync.dma_start(xb[:], xbblob[ti, :, 0:KC * nt].rearrange(
            "p (kc t) -> p kc t", kc=KC))
        if first:
            load_main_weights()
        ind = sp.tile([tl["nb"], nt], BF16, tag="ind")
        nc.gpsimd.dma_start(ind[:], bass.AP(tensor=ind_blob.tensor,
                                            offset=ind_blob.offset + ind_offs[ti],
                                            ap=[[nt, tl["nb"]], [1, nt]]))

        # mm1: h1 = relu((16*W1).T @ x8) / 16  [+ b1]
        h1 = hp.tile([128, KH, nt], F8, tag="h1")
        for g in range(KH // 2):
            ps = psum.tile([128, 2, 512], F32, tag="mm", bufs=3)
            for j in range(2):
                mc = 2 * g + j
                sl = slice(mc * 128, (mc + 1) * 128)
                for kk in range(0, KC, 2):
                    nc.tensor.matmul(ps[:, j, 0:nt], wtiles["W_mi1"][:, kk:kk + 2, sl],
                                     x8[:, kk:kk + 2, :], perf_mode=DR,
                                     start=(kk == 0),
                                     stop=(kk == KC - 2 and not has_b1),
                                     skip_group_check=True)
                if has_b1:
                    # b1_row is pre-scaled by WSC on the host
                    nc.tensor.matmul(ps[:, j, 0:nt], wtiles["b1_row"][0:1, sl],
                                     ones_row[0:1, 0:nt],
                                     start=False, stop=True,
                                     skip_group_check=True)
            # batched eviction over 2 banks; scale compensates the 16x weights
            nc.scalar.activation(h1[:, 2 * g:2 * g + 2, :], ps[:, :, 0:nt],
                                 AF.Relu, scale=1.0 / WSC)

        # mm2: r = W2.T @ h1 + (x + b2)   (W2 raw fp8, vector residual add)
        r_bf = rp.tile([128, KC, nt], BF16, tag="r")
        for g in range(KC // 2):
            ps = psum.tile([128, 2, 512], F32, tag="mm", bufs=3)
            for j in range(2):
                mc = 2 * g + j
                sl = slice(mc * 128, (mc + 1) * 128)
                for kk in range(0, KH, 2):
                    nc.tensor.matmul(ps[:, j, 0:nt], wtiles["W_mi2"][:, kk:kk + 2, sl],
                                     h1[:, kk:kk + 2, :], perf_mode=DR,
                                     start=(kk == 0), stop=(kk == KH - 2),
                                     skip_group_check=True)
            nc.vector.tensor_add(r_bf[:, 2 * g:2 * g + 2, :], ps[:, :, 0:nt],
                                 xb[:, 2 * g:2 * g + 2, :])
        return x8, xb, ind, r_bf

    def stage2(ti, s1):
        """attention + sc GEMM + f1 GEMM -> out tile store."""
        tl = tiles[ti]
        nt = _pad16(tl["nt"])
        nb, b0 = tl["nb"], tl["b0"]
        x8, xb, ind, r_bf = s1
        # qw_scr is written by qstage_b; stage2 is always emitted after it,
        # so this read is RAW-ordered correctly
        qw_loc = sp.tile([nb, D], BF16, tag="qwloc")
        nc.gpsimd.dma_start(qw_loc[:], qw_scr[b0:b0 + nb, :])

        # block-diag attention scores (bf16)
        at = psum.tile([nb, nt], F32, tag="at", bufs=1)
        for kc in range(KC):
            nc.tensor.matmul(at[:], qT_bf[:, kc, b0:b0 + nb], r_bf[:, kc, :],
                             start=(kc == 0), stop=(kc == KC - 1))
        masked = sp.tile([nb, nt], BF16, tag="msk")
        nc.vector.tensor_tensor(masked[:], at[:], ind[:], op=ALU.mult)
        # fused reduce-over-rows + broadcast-to-128-partitions
        wps = psum.tile([128, nt], F32, tag="w", bufs=1)
        nc.tensor.matmul(wps[:], ones_sq[0:nb, :], masked[:], start=True, stop=True)
        w_bc = sp.tile([128, nt], BF16, tag="wbc")
        nc.scalar.activation(w_bc[:], wps[:], AF.Sigmoid,
                             scale=float(1.0 / np.sqrt(D)))

        # wc = w * r  (fp8 for the sc GEMM)
        wc8 = wcp.tile([128, KC, nt], F8, tag="wc")
        for kc in range(KC):
            nc.vector.tensor_mul(wc8[:, kc, :], r_bf[:, kc, :], w_bc[:])

        # mm3: scaling = tanh((16*sc_W).T @ wc / 16  [+ sc_b])
        scal = scp.tile([128, KC, nt], BF16, tag="scal")
        for g in range(KC // 2):
            ps = psum.tile([128, 2, 512], F32, tag="mm", bufs=3)
            for j in range(2):
                mc = 2 * g + j
                sl = slice(mc * 128, (mc + 1) * 128)
                for kk in range(0, KC, 2):
                    nc.tensor.matmul(ps[:, j, 0:nt], wtiles["W_sc"][:, kk:kk + 2, sl],
                                     wc8[:, kk:kk + 2, :], perf_mode=DR,
                                     start=(kk == 0),
                                     stop=(kk == KC - 2 and not has_scb),
                                     skip_group_check=True)
                if has_scb:
                    nc.tensor.matmul(ps[:, j, 0:nt], wtiles["scb_row"][0:1, sl],
                                     ones_row[0:1, 0:nt],
                                     start=False, stop=True,
                                     skip_group_check=True)
            nc.scalar.activation(scal[:, 2 * g:2 * g + 2, :], ps[:, :, 0:nt],
                                 AF.Tanh, scale=1.0 / WSC)

        # z = wc * scaling (fp8); mm4: out = relu((16*f1_W).T @ z + 16*qw)/16
        z8 = zp.tile([128, KC, nt], F8, tag="z")
        for g in range(2):
            nc.vector.tensor_mul(z8[:, 4 * g:4 * g + 4, :],
                                 wc8[:, 4 * g:4 * g + 4, :],
                                 scal[:, 4 * g:4 * g + 4, :])
        o_bf = op.tile([128, KC, nt], BF16, tag="o")
        for g in range(KC // 2):
            ps = psum.tile([128, 2, 512], F32, tag="mm", bufs=3)
            for j in range(2):
                mc = 2 * g + j
                sl = slice(mc * 128, (mc + 1) * 128)
                for kk in range(0, KC, 2):
                    nc.tensor.matmul(ps[:, j, 0:nt], wtiles["W_f1"][:, kk:kk + 2, sl],
                                     z8[:, kk:kk + 2, :], perf_mode=DR,
                                     start=(kk == 0), stop=False,
                                     skip_group_check=True)
                nc.tensor.matmul(ps[:, j, 0:nt], qw_loc[:, sl], ind[:],
                                 start=False, stop=True, skip_group_check=True)
            nc.scalar.activation(o_bf[:, 2 * g:2 * g + 2, :], ps[:, :, 0:nt],
                                 AF.Relu, scale=1.0 / WSC)
        nc.sync.dma_start(
            outblob[ti, :, 0:KC * nt].rearrange("p (kc t) -> p kc t", kc=KC),
            o_bf[:])

    # ======================= q-stage (two parts) =========================
    qp = tc.tile_pool(name="qpool", bufs=1)
    qpc = qp.__enter__()

    def qstage_loads():
        """issue q-stage input DMAs early (Pool queue) so unet lands first."""
        unet_sb = qpc.tile([BC, SEG_C, 49], BF16)
        nc.gpsimd.dma_start(unet_sb[:], unet[:, :, :])
        W_seg_a = qpc.tile([128, D], BF16)
        nc.gpsimd.dma_start(W_seg_a[:], wi["seg_W"][0:128, :])
        W_seg_b = qpc.tile([5, D], BF16)
        nc.gpsimd.dma_start(W_seg_b[:], wi["seg_W"][128:SEG_C, :])
        b_segr = qpc.tile([1, D], BF16)
        nc.gpsimd.dma_start(b_segr[:], wi["seg_b"])
        g_bc = qpc.tile([128, D], BF16)
        nc.gpsimd.dma_start(g_bc[:], bass.AP(tensor=wi["ln_g"].tensor,
                                             offset=wi["ln_g"].offset,
                                             ap=[[0, 128], [1, D]]))
        bb_bc = qpc.tile([128, D], BF16)
        nc.gpsimd.dma_start(bb_bc[:], bass.AP(tensor=wi["ln_b"].tensor,
                                              offset=wi["ln_b"].offset,
                                              ap=[[0, 128], [1, D]]))
        return unet_sb, W_seg_a, W_seg_b, b_segr, g_bc, bb_bc

    def qstage_a(loads):
        """unet pooling + seg embedding + layernorm -> qn_bf (token-major)."""
        unet_sb, W_seg_a, W_seg_b, b_segr, g_bc, bb_bc = loads

        pooled = qpc.tile([BC, SEG_C], F32)
        nc.vector.reduce_sum(pooled[:], unet_sb[:], axis=mybir.AxisListType.X)
        pooled_bf = qpc.tile([BC, SEG_C], BF16)
        nc.scalar.mul(pooled_bf[:], pooled[:], 1.0 / 49.0)
        pa_ps = psum.tile([128, BC], BF16, tag="w", bufs=1)
        nc.tensor.transpose(pa_ps[:], pooled_bf[:, 0:128], ident_bf[0:BC, 0:BC])
        pa_bf = qpc.tile([128, BC], BF16)
        nc.scalar.copy(pa_bf[:], pa_ps[:])
        pb_ps = psum.tile([5, BC], BF16, tag="w", bufs=1)
        nc.tensor.transpose(pb_ps[:], pooled_bf[:, 128:SEG_C], ident_bf[0:BC, 0:BC])
        pb_bf = qpc.tile([5, BC], BF16)
        nc.scalar.copy(pb_bf[:], pb_ps[:])

        # q1 = relu(pooled @ seg_W + seg_b)   (token-major: BC x D)
        q1 = qpc.tile([BC, D], F32)
        for ng in range(2):
            sl = slice(ng * 512, (ng + 1) * 512)
            ps = psum.tile([BC, 512], F32, tag="mm", bufs=3)
            nc.tensor.matmul(ps[:], pa_bf[:], W_seg_a[:, sl], start=True, stop=False)
            nc.tensor.matmul(ps[:], pb_bf[:], W_seg_b[:, sl], start=False, stop=False)
            nc.tensor.matmul(ps[:], ones_row[0:1, 0:BC], b_segr[0:1, sl],
                             start=False, stop=True)
            nc.vector.tensor_scalar_max(q1[:, sl], ps[:], 0.0)

        # layernorm over D
        stats = qpc.tile([BC, 2, 6], F32)
        for s in range(2):
            nc.vector.bn_stats(stats[:, s, :], q1[:, s * 512:(s + 1) * 512])
        mv = qpc.tile([BC, 2], F32)
        nc.vector.bn_aggr(mv[:], stats[:])
        rstd = qpc.tile([BC, 1], F32)
        nc.scalar.activation(rstd[:], mv[:, 1:2], AF.Sqrt, bias=eps_t[0:BC, :])
        nc.vector.reciprocal(rstd[:], rstd[:])
        qn = qpc.tile([BC, D], F32)
        nc.vector.tensor_scalar(qn[:], q1[:], mv[:, 0:1], rstd[:],
                                op0=ALU.subtract, op1=ALU.mult)
        nc.vector.tensor_mul(qn[:], qn[:], g_bc[0:BC, :])
        qn_bf = qpc.tile([BC, D], BF16)
        nc.vector.tensor_add(qn_bf[:], qn[:], bb_bc[0:BC, :])
        return qn_bf

    def qstage_b(qn_bf):
        """q MLP (bf16) -> qT_bf; qw = 16*(q @ f1_W + f1_b) -> qw_scr."""
        W_ms1 = qpc.tile([128, KC, H], BF16)
        nc.gpsimd.dma_start(W_ms1[:], wi["ms_W1"].rearrange("p (kc m) -> p kc m", kc=KC))
        W_ms2 = qpc.tile([128, KH, D], BF16)
        nc.gpsimd.dma_start(W_ms2[:], wi["ms_W2"].rearrange("p (kc m) -> p kc m", kc=KH))
        b_ms1r = qpc.tile([1, H], BF16)
        nc.gpsimd.dma_start(b_ms1r[:], wi["ms_b1"])
        b_ms2r = qpc.tile([1, D], BF16)
        nc.gpsimd.dma_start(b_ms2r[:], wi["ms_b2"])
        fb_bc = qpc.tile([BC, D], BF16)
        nc.gpsimd.dma_start(fb_bc[:], bass.AP(tensor=wi["f1_b"].tensor,
                                              offset=wi["f1_b"].offset,
                                              ap=[[0, BC], [1, D]]))

        # qnT (feature-major) via PE transposes
        qnT_bf = qpc.tile([128, KC, BC], BF16)
        for kc in range(KC):
            pt = psum.tile([128, BC], BF16, tag="w", bufs=1)
            nc.tensor.transpose(pt[:], qn_bf[:, kc * 128:(kc + 1) * 128],
                                ident_bf[0:BC, 0:BC])
            nc.scalar.copy(qnT_bf[:, kc, :], pt[:])

        # q MLP (feature-major): qm = relu(ms_W1.T @ qnT + b1)
        qmT_bf = qpc.tile([128, KH, BC], BF16)
        for mc in range(KH):
            sl = slice(mc * 128, (mc + 1) * 128)
            ps = psum.tile([128, BC], F32, tag="mm", bufs=3)
            for kc in range(KC):
                nc.tensor.matmul(ps[:], W_ms1[:, kc, sl], qnT_bf[:, kc, :],
                                 start=(kc == 0), stop=False)
            nc.tensor.matmul(ps[:], b_ms1r[0:1, sl], ones_row[0:1, 0:BC],
                             start=False, stop=True)
            nc.scalar.activation(qmT_bf[:, mc, :], ps[:], AF.Relu)
        # q2T = ms_W2.T @ qmT + b2 + qnT   -> qT_bf
        for mc in range(KC):
            sl = slice(mc * 128, (mc + 1) * 128)
            ps = psum.tile([128, BC], F32, tag="mm", bufs=3)
            for kc in range(KH):
                nc.tensor.matmul(ps[:], W_ms2[:, kc, sl], qmT_bf[:, kc, :],
                                 start=(kc == 0), stop=False)
            nc.tensor.matmul(ps[:], b_ms2r[0:1, sl], ones_row[0:1, 0:BC],
                             start=False, stop=True)
            nc.vector.tensor_add(qT_bf[:, mc, :], ps[:], qnT_bf[:, mc, :])

        # qw = 16*(q2 @ f1_W + f1_b) (token-major); DRAM scratch for row slices
        qw_bf = qpc.tile([BC, D], BF16)
        for ng in range(2):
            sl = slice(ng * 512, (ng + 1) * 512)
            W_f1h = qpc.tile([128, KC, 512], BF16, tag="wf1h", bufs=1)
            nc.gpsimd.dma_start(
                W_f1h[:], wi["f1_Wb"][ng].rearrange("p (kc m) -> p kc m", kc=KC))
            ps = psum.tile([BC, 512], F32, tag="mm", bufs=3)
            for kc in range(KC):
                nc.tensor.matmul(ps[:], qT_bf[:, kc, :], W_f1h[:, kc, :],
                                 start=(kc == 0), stop=(kc == KC - 1))
            nc.vector.tensor_add(qw_bf[:, sl], ps[:], fb_bc[0:BC, sl])
        nc.scalar.mul(qw_bf[:], qw_bf[:], WSC)
        nc.gpsimd.dma_start(qw_scr[:, :], qw_bf[:])

    # ======================= emission order ==============================
    qloads = qstage_loads()
    s1_results = {}
    s1_results[0] = stage1(0, first=True)
    qn_bf = qstage_a(qloads)
    if ntiles > 1:
        s1_results[1] = stage1(1)
    qstage_b(qn_bf)
    qp.__exit__(None, None, None)

    for ti in range(ntiles):
        stage2(ti, s1_results.pop(ti))
        nxt = ti + 2
        if nxt < ntiles:
            s1_results[nxt] = stage1(nxt)


def _build(plan, has_b1, has_scb):
    nc = bacc.Bacc("TRN2", target_bir_lowering=False, debug=False)
    ctx = ExitStack()
    with tile.TileContext(nc) as tc, ctx:
        _emit(ctx, tc, plan, has_b1, has_scb)
    nc.compile()
    return nc


_NC_CACHE = {}


def _get_nc(plan_key, plan, has_b1, has_scb):
    if plan_key not in _NC_CACHE:
        _NC_CACHE[plan_key] = _build(plan, has_b1, has_scb)
    return _NC_CACHE[plan_key]


def _build_ind_blob(tiles):
    sz = sum(t["nb"] * _pad16(t["nt"]) for t in tiles)
    blob = np.zeros(sz, dtype=NPBF16)
    off = 0
    for t in tiles:
        ntp = _pad16(t["nt"])
        ind = np.zeros((t["nb"], ntp), dtype=NPBF16)
        for row, lo, hi in t["segs"]:
            ind[row, lo:hi] = 1
        blob[off:off + ind.size] = ind.ravel()
        off += ind.size
    return blob


def _run_cores(ncs, in_maps, trace=False):
    """Dispatch one compiled program per core, concurrently."""
    import jax
    from concourse import bass2jax
    from concourse.bass2jax import _bass_exec_p, install_neuronx_cc_hook

    install_neuronx_cc_hook()
    devices = jax.devices()[:NCORES]

    def make_jit(nc):
        in_names, out_names, out_avals, zero_outs = [], [], [], []
        for alloc in nc.m.functions[0].allocations:
            if not isinstance(alloc, mybir.MemoryLocationSet):
                continue
            name = alloc.memorylocations[0].name
            if alloc.kind == "ExternalInput":
                in_names.append(name)
            elif alloc.kind == "ExternalOutput":
                out_names.append(name)
                shape = tuple(alloc.tensor_shape)
                dtype = mybir.dt.np(alloc.dtype)
                out_avals.append(jax.core.ShapedArray(shape, dtype))
                zero_outs.append(np.zeros(shape, dtype))
        n_params = len(in_names)
        all_names = in_names + out_names

        def _body(*args):
            outs = _bass_exec_p.bind(
                *args,
                out_avals=tuple(out_avals),
                in_names=tuple(all_names),
                out_names=tuple(out_names),
                lowering_input_output_aliases=(),
                sim_require_finite=True,
                sim_require_nnan=True,
                nc=nc,
            )
            return tuple(outs)

        donate = tuple(range(n_params, n_params + len(out_names)))
        return (jax.jit(_body, donate_argnums=donate, keep_unused=True),
                in_names, out_names, zero_outs)

    with ThreadPoolExecutor(NCORES) as ex:
        jits = list(ex.map(make_jit, ncs))

    def launch(c):
        jitted, in_names, out_names, zero_outs = jits[c]
        vals = dict(in_maps[c])
        pid = ncs[c].partition_id_tensor
        if pid is not None:
            vals[pid.name] = np.array([[c]], dtype=np.uint32)
        args = [jax.device_put(np.asarray(vals[n]), devices[c]) for n in in_names]
        zz = [jax.device_put(z, devices[c]) for z in zero_outs]
        outs = jitted(*args, *zz)
        return dict(zip(out_names, outs))

    def run_all():
        with ThreadPoolExecutor(NCORES) as ex:
            outs = list(ex.map(launch, range(NCORES)))
        return [{k: np.asarray(v) for k, v in o.items()} for o in outs]

    global LAST_EXEC_NS, _LAST_TRACE
    if trace:
        import glob as globmod
        import tempfile
        from antenv.axon_hooks import get_axon_ntff_profile_hook
        hook = get_axon_ntff_profile_hook()
        neff_dir = tempfile.mkdtemp()
        if hook is None:
            results = run_all()
        else:
            run_all()  # warm: jit trace + NEFF compile before the profiled run
            with hook(neff_dir, [0]):
                results = run_all()
            try:
                import re
                import shutil
                import gauge.profiler
                from concourse._compat import FishPath
                ntffs = sorted(globmod.glob(os.path.join(neff_dir, "*_body*.ntff")))
                times = []
                insts_best = None
                for ntff in ntffs:
                    m = re.search(r"executable(\d+)", os.path.basename(ntff))
                    exe = m.group(1)
                    sub = os.path.join(neff_dir, f"exe{exe}")
                    os.makedirs(sub, exist_ok=True)
                    for fpath in globmod.glob(os.path.join(neff_dir, f"*executable{exe}*")):
                        if os.path.isfile(fpath):
                            shutil.copy(fpath, sub)
                    profile = gauge.profiler.Profile(
                        profile_path=FishPath(sub), kernel_dev_mode=True,
                        profile_on_exit=False, bass_kernel=ncs[0].m,
                        offline_processing=True, fname="*_body*",
                        metadata={"artifacts_path": sub})
                    pr = profile.to_perfetto(model_index=(0,))
                    if pr:
                        times.append(pr[0].exec_time_ns)
                        if pr[0].exec_time_ns == max(times):
                            insts_best = (pr[0].insts, pr[0].trace_path)
                if times:
                    LAST_EXEC_NS = max(times)
                    _LAST_TRACE = insts_best
                    print(f"per-core exec ns: {sorted(times)}", file=sys.stderr)
                    print(f"neff_dir: {neff_dir}", file=sys.stderr)
            except Exception as e:
                print(f"profile post-processing failed: {e!r}", file=sys.stderr)
    else:
        results = run_all()
    return results


def kernel(rgns, Unet_segs, region_lens, mi_W1, mi_b1, mi_W2, mi_b2,
           ms_W1, ms_b1, ms_W2, ms_b2, seg_W, seg_b, ln_g, ln_b,
           sc_W, sc_b, f1_W, f1_b):
    _wire_ntff_hook()

    f = lambda a: np.ascontiguousarray(np.asarray(a, dtype=np.float32))
    bf = lambda a: np.ascontiguousarray(np.asarray(a, dtype=np.float32).astype(NPBF16))
    f8s = lambda a: np.ascontiguousarray(
        (np.asarray(a, dtype=np.float32) * WSC).astype(NPF8))
    f8r = lambda a: np.ascontiguousarray(np.asarray(a, dtype=np.float32).astype(NPF8))
    rgns = f(rgns)
    unet = bf(Unet_segs).reshape(B, SEG_C, 49)
    lens = np.clip(np.asarray(region_lens).astype(np.int64), 0, R)

    def pack(w, kchunks):
        w = np.asarray(w)
        m = w.shape[1]
        return np.ascontiguousarray(
            w.reshape(kchunks, 128, m).transpose(1, 0, 2).reshape(128, kchunks * m))

    mi_b1_ = f(mi_b1).reshape(1, H)
    sc_b_ = f(sc_b).reshape(1, D)
    has_b1 = bool(np.any(mi_b1_ != 0))
    has_scb = bool(np.any(sc_b_ != 0))

    f1_Wb_ = bf(f1_W)                      # (D, D) bf16
    f1_Wb_halves = np.stack([pack(f1_Wb_[:, 0:512], KC),
                             pack(f1_Wb_[:, 512:1024], KC)])
    weights = {
        "mi_W1": pack(f8s(mi_W1), KC), "mi_b1": (mi_b1_ * WSC).astype(NPBF16),
        "mi_W2": pack(f8r(mi_W2), KH),
        "ms_W1": pack(bf(ms_W1), KC), "ms_b1": bf(ms_b1).reshape(1, H),
        "ms_W2": pack(bf(ms_W2), KH), "ms_b2": bf(ms_b2).reshape(1, D),
        "seg_W": bf(seg_W), "seg_b": bf(seg_b).reshape(1, D),
        "ln_g": bf(ln_g).reshape(1, D), "ln_b": bf(ln_b).reshape(1, D),
        "sc_W": pack(f8s(sc_W), KC), "sc_b": (sc_b_ * WSC).astype(NPBF16),
        "f1_W": pack(f8s(f1_W), KC), "f1_Wb": f1_Wb_halves,
        "f1_b": bf(f1_b).reshape(1, D),
    }

    # balanced batch assignment: 128 batches per core, equalize token counts
    order = np.argsort(-lens, kind="stable")
    loads = np.zeros(NCORES, dtype=np.int64)
    counts = np.zeros(NCORES, dtype=np.int64)
    assign = [[] for _ in range(NCORES)]
    for b in order:
        open_cores = [c for c in range(NCORES) if counts[c] < BC]
        c = min(open_cores, key=lambda c: loads[c])
        assign[c].append(int(b))
        loads[c] += int(lens[b])
        counts[c] += 1
    batches = [np.sort(np.array(a, dtype=np.int64)) for a in assign]

    rflat = rgns.reshape(B * R, D)
    b2row = f(mi_b2).reshape(1, D)
    in_maps, plans, vrows, tileinfo = [], [], [], []
    for c in range(NCORES):
        bl = batches[c]
        lens_c = lens[bl]
        plan = _make_plan(lens_c)
        plans.append(plan)
        tiles = plan[0]
        ntiles = len(tiles)
        rows = np.concatenate([bl[i] * R + np.arange(lens_c[i]) for i in range(BC)])
        vrows.append(rows)
        xv = rflat[rows]                      # (ntokc, D) f32
        x8b = np.zeros((ntiles, 128, KC * TOKCAP), dtype=NPF8)
        xbb = np.zeros((ntiles, 128, KC * TOKCAP), dtype=NPBF16)
        xvb2 = (xv + b2row).astype(NPBF16)
        xv8 = xv.astype(NPF8)
        for ti, tl in enumerate(tiles):
            t0, nt = tl["t0"], tl["nt"]
            ntp = _pad16(nt)
            # (nt, D) -> (128, KC, ntp)
            blk8 = xv8[t0:t0 + nt].reshape(nt, KC, 128).transpose(2, 1, 0)
            blkb = xvb2[t0:t0 + nt].reshape(nt, KC, 128).transpose(2, 1, 0)
            x8b[ti, :, :KC * ntp].reshape(128, KC, ntp)[:, :, :nt] = blk8
            xbb[ti, :, :KC * ntp].reshape(128, KC, ntp)[:, :, :nt] = blkb
        tileinfo.append(tiles)
        in_maps.append(dict(
            x8blob=x8b,
            xbblob=xbb,
            unet=np.ascontiguousarray(unet[bl]),
            ind=_build_ind_blob(tiles),
            **weights,
        ))

    def plan_key(c):
        return tuple((t["t0"], t["nt"], t["b0"], t["nb"], tuple(t["segs"]))
                     for t in plans[c][0])

    keys = [plan_key(c) for c in range(NCORES)]
    uniq = {}
    for c in range(NCORES):
        if keys[c] not in uniq:
            uniq[keys[c]] = None
    with ThreadPoolExecutor(min(8, len(uniq))) as ex:
        built = dict(zip(uniq.keys(),
                         ex.map(lambda k: _get_nc(k, plans[keys.index(k)],
                                                  has_b1, has_scb),
                                list(uniq.keys()))))
    ncs = [built[keys[c]] for c in range(NCORES)]

    trace = bool(int(os.environ.get("BASSK_TRACE", "0")))
    results = _run_cores(ncs, in_maps, trace=trace)

    out = np.empty((B * R, D), np.float32)
    out[:] = np.maximum(f(f1_b).reshape(1, D), 0.0)
    for c in range(NCORES):
        ob = results[c]["outblob"]            # (ntiles, 128, KC*TOKCAP) bf16
        tiles = tileinfo[c]
        ntokc = plans[c][1]
        res = np.empty((ntokc, D), np.float32)
        for ti, tl in enumerate(tiles):
            t0, nt = tl["t0"], tl["nt"]
            ntp = _pad16(nt)
            blk = ob[ti, :, :KC * ntp].reshape(128, KC, ntp)[:, :, :nt]
            res[t0:t0 + nt] = blk.transpose(2, 1, 0).reshape(nt, D).astype(np.float32)
        out[vrows[c]] = res
    return out.reshape(B, R, D)


# revision 21
# speedup vs baseline: 1.0103x; 1.0103x over previous
"""Trainium2 Bass kernel for nn_CrossmodalFusion (B=1024, R=36, D=1024).

Data-parallel over batch across 8 NeuronCores with token-level sparsity:
tokens with j >= region_lens[b] produce exactly relu(f1_b), so the host
compacts each core's valid tokens (~51%), the device processes only those,
and the host scatters results back (masked rows filled with host-computed
relu(f1_b), which is exact).

Device layout is feature-major (features on SBUF partitions, tokens on the
free dim); every GEMM uses small replicated weights as the stationary lhsT.
The four big GEMMs (MLP1, MLP2, sc_W, f1_W) run in fp8 e4m3 with
perf_mode=DoubleRow (2 K-chunks per instruction, ~1.5x PE throughput);
weights are pre-scaled by 16 (power of two) on the host and compensated
exactly in the PSUM evictions to keep small weights out of the fp8
subnormal range. The residual path stays bf16: the host ships both
x_fp8 (for MLP input) and bf16(x + mi_b2) (residual, folded bias).

Attention is a block-diagonal bf16 matmul + indicator-mask + a single
fused ones-matmul that simultaneously reduces over the batch rows and
broadcasts the per-token score to all 128 partitions (no DRAM bounce).
seg_rep is folded into the last GEMM via qw = 16*(q @ f1_W + f1_b) rows
gathered with the indicator matrix.

Inputs/outputs stream as per-tile packed blobs (contiguous 4-8KB per
partition row); outputs return in bf16 and are upcast on the host.
DMA traffic is spread over the three issue queues (SP / Activation /
Pool) so weight, x-tile and unet transfers flow in parallel and the PE
starts within a few microseconds.

Because each core's token plan differs, 8 per-core programs are compiled
(concurrently) and dispatched asynchronously, one per NeuronCore.
"""
import os
import sys
import types
from concurrent.futures import ThreadPoolExecutor
from contextlib import ExitStack

sys.path.insert(0, "/opt/trn_rl_repo")

import numpy as np
import ml_dtypes

import concourse.bass as bass
import concourse.tile as tile
from concourse import bacc, mybir
from concourse.masks import make_identity

F32 = mybir.dt.float32
BF16 = mybir.dt.bfloat16
F8 = mybir.dt.float8e4

NPBF16 = ml_dtypes.bfloat16
NPF8 = ml_dtypes.float8_e4m3

B, R, D = 1024, 36, 1024
H = D // 2
SEG_C = 133
NCORES = 8
BC = B // NCORES            # batches per core
KC = D // 128               # 8 feature chunks
KH = H // 128               # 4 hidden chunks

TOKCAP = 512                # tokens per tile (PSUM fp32 bank width)
WSC = 16.0                  # fp8 weight pre-scale (power of 2, exact)

LAST_EXEC_NS = None
_LAST_TRACE = None

DR = mybir.MatmulPerfMode.DoubleRow


def _wire_ntff_hook():
    if "antenv.axon_hooks" in sys.modules:
        return
    try:
        import trn_agent_boot.trn_boot as tb
        hook = tb._ntff_profile_via_ctypes("/opt/axon/libaxon_pjrt.so")
    except Exception:
        hook = None
    mod = types.ModuleType("antenv.axon_hooks")
    _h = [hook]
    mod.set_axon_ntff_profile_hook = lambda h: _h.__setitem__(0, h)
    mod.get_axon_ntff_profile_hook = lambda: _h[0]
    sys.modules["antenv.axon_hooks"] = mod


def _pad16(n):
    return (n + 15) // 16 * 16


def _make_plan(lens_c):
    """Tile plan for one core from its per-batch valid-token counts."""
    stream = []  # (local_batch, j)
    for lb, ln in enumerate(lens_c):
        stream.extend((lb, j) for j in range(int(ln)))
    ntokc = len(stream)
    tiles = []
    t0 = 0
    while t0 < ntokc:
        nt = 0
        b_first = stream[t0][0]
        while t0 + nt < ntokc and nt < TOKCAP:
            lb = stream[t0 + nt][0]
            if lb - b_first + 1 > 128:
                break
            nt += 1
        b_last = stream[t0 + nt - 1][0]
        segs = []
        pos = 0
        while pos < nt:
            lb = stream[t0 + pos][0]
            end = pos
            while end < nt and stream[t0 + end][0] == lb:
                end += 1
            segs.append((lb - b_first, pos, end))
            pos = end
        tiles.append(dict(t0=t0, nt=nt, b0=b_first, nb=b_last - b_first + 1, segs=segs))
        t0 += nt
    return tiles, ntokc


def _emit(ctx, tc, plan, has_b1, has_scb):
    nc = tc.nc
    AF = mybir.ActivationFunctionType
    ALU = mybir.AluOpType
    tiles, ntokc = plan
    ntiles = len(tiles)

    # ---- DRAM I/O -------------------------------------------------------
    x8blob = nc.dram_tensor("x8blob", [ntiles, 128, KC * TOKCAP], F8,
                            kind="ExternalInput").ap()
    xbblob = nc.dram_tensor("xbblob", [ntiles, 128, KC * TOKCAP], BF16,
                            kind="ExternalInput").ap()
    unet = nc.dram_tensor("unet", [BC, SEG_C, 49], BF16, kind="ExternalInput").ap()
    ind_sz = sum(t["nb"] * _pad16(t["nt"]) for t in tiles)
    ind_blob = nc.dram_tensor("ind", [ind_sz], BF16, kind="ExternalInput").ap()
    wi = {}
    for name, shape, dt in [
        ("mi_W1", [128, KC * H], F8), ("mi_b1", [1, H], BF16),
        ("mi_W2", [128, KH * D], F8),
        ("ms_W1", [128, KC * H], BF16), ("ms_b1", [1, H], BF16),
        ("ms_W2", [128, KH * D], BF16), ("ms_b2", [1, D], BF16),
        ("seg_W", [SEG_C, D], BF16), ("seg_b", [1, D], BF16),
        ("ln_g", [1, D], BF16), ("ln_b", [1, D], BF16),
        ("sc_W", [128, KC * D], F8), ("sc_b", [1, D], BF16),
        ("f1_W", [128, KC * D], F8), ("f1_Wb", [2, 128, KC * 512], BF16),
        ("f1_b", [1, D], BF16),
    ]:
        wi[name] = nc.dram_tensor(name, shape, dt, kind="ExternalInput").ap()
    outblob = nc.dram_tensor("outblob", [ntiles, 128, KC * TOKCAP], BF16,
                             kind="ExternalOutput").ap()
    qw_scr = nc.dram_tensor("qw_scr", [BC, D], BF16).ap()

    # ---- persistent constants ------------------------------------------
    const = ctx.enter_context(tc.tile_pool(name="const", bufs=1))

    def load_w(name, kchunks, m, dt, eng):
        # host pre-packs to [128, kc*m]: per-partition rows fully contiguous
        t = const.tile([128, kchunks, m], dt, tag=f"cw_{name}")
        eng.dma_start(t[:], wi[name].rearrange("p (kc m) -> p kc m", kc=kchunks))
        return t

    # main-loop fp8 weights go on the SP (sync) queue, issued between the
    # tile-0 x loads and tile-1 loads, keeping the scalar queue compute-only
    wtiles = {}

    def load_main_weights():
        # mm1/mm2 weights: Activation HWDGE queue (first two issues, parallel
        # with the x loads on SP); later-needed weights: Pool queue
        wtiles["W_mi1"] = load_w("mi_W1", KC, H, F8, nc.scalar)
        wtiles["W_mi2"] = load_w("mi_W2", KH, D, F8, nc.scalar)
        wtiles["W_sc"] = load_w("sc_W", KC, D, F8, nc.gpsimd)
        wtiles["W_f1"] = load_w("f1_W", KC, D, F8, nc.gpsimd)
        b1 = const.tile([1, H], BF16, tag="b1row")
        nc.gpsimd.dma_start(b1[:], wi["mi_b1"])
        wtiles["b1_row"] = b1
        scb = const.tile([1, D], BF16, tag="scbrow")
        nc.gpsimd.dma_start(scb[:], wi["sc_b"])
        wtiles["scb_row"] = scb

    ones_row = const.tile([1, 512], BF16)
    nc.vector.memset(ones_row[:], 1.0)
    ones_sq = const.tile([128, 128], BF16)
    nc.vector.memset(ones_sq[:], 1.0)
    ident_bf = const.tile([128, 128], BF16)
    make_identity(nc, ident_bf)
    eps_t = const.tile([128, 1], F32)
    nc.vector.memset(eps_t[:], 1e-5)

    qT_bf = const.tile([128, KC, BC], BF16)    # feature-major q (lhsT for attn)

    psum = ctx.enter_context(tc.tile_pool(name="psum", bufs=1, space="PSUM"))

    # ---- main-loop pools ------------------------------------------------
    xp8 = ctx.enter_context(tc.tile_pool(name="xp8", bufs=2))
    xpb = ctx.enter_context(tc.tile_pool(name="xpb", bufs=2))
    hp = ctx.enter_context(tc.tile_pool(name="hp", bufs=2))
    rp = ctx.enter_context(tc.tile_pool(name="rp", bufs=2))
    wcp = ctx.enter_context(tc.tile_pool(name="wcp", bufs=2))
    scp = ctx.enter_context(tc.tile_pool(name="scp", bufs=2))
    zp = ctx.enter_context(tc.tile_pool(name="zp", bufs=2))
    op = ctx.enter_context(tc.tile_pool(name="op", bufs=2))
    sp = ctx.enter_context(tc.tile_pool(name="sp", bufs=2))

    ind_offs = []
    off = 0
    for t in tiles:
        ind_offs.append(off)
        off += t["nb"] * _pad16(t["nt"])

    def stage1(ti):
        """x loads + MLP (mm1 fp8 DR, mm2 fp8 DR + bf16 residual) -> r_bf."""
        tl = tiles[ti]
        nt = _pad16(tl["nt"])
        x8 = xp8.tile([128, KC, nt], F8, tag="x8")
        nc.sync.dma_start(x8[:], x8blob[ti, :, 0:KC * nt].rearrange(
            "p (kc t) -> p kc t", kc=KC))
        xb = xpb.tile([128, KC, nt], BF16, tag="xb")
        nc.sync.dma_start(xb[:], xbblob[ti, :, 0:KC * nt].rearrange(
            "p (kc t) -> p kc t", kc=KC))
        ind = sp.tile([tl["nb"], nt], BF16, tag="ind")
        nc.gpsimd.dma_start(ind[:], bass.AP(tensor=ind_blob.tensor,
                                            offset=ind_blob.offset + ind_offs[ti],
                                            ap=[[nt, tl["nb"]], [1, nt]]))

        # mm1: h1 = relu((16*W1).T @ x8) / 16  [+ b1]
        h1 = hp.tile([128, KH, nt], F8, tag="h1")
        for g in range(KH // 2):
            ps = psum.tile([128, 2, 512], F32, tag="mm", bufs=3)
            for j in range(2):
                mc = 2 * g + j
                sl = slice(mc * 128, (mc + 1) * 128)
                for kk in range(0, KC, 2):
                    nc.tensor.matmul(ps[:, j, 0:nt], wtiles["W_mi1"][:, kk:kk + 2, sl],
                                     x8[:, kk:kk + 2, :], perf_mode=DR,
                                     start=(kk == 0),
                                     stop=(kk == KC - 2 and not has_b1),
                                     skip_group_check=True)
                if has_b1:
                    # b1_row is pre-scaled by WSC on the host
                    nc.tensor.matmul(ps[:, j, 0:nt], wtiles["b1_row"][0:1, sl],
                                     ones_row[0:1, 0:nt],
                                     start=False, stop=True,
                                     skip_group_check=True)
            # batched eviction over 2 banks; scale compensates the 16x weights
            nc.scalar.activation(h1[:, 2 * g:2 * g + 2, :], ps[:, :, 0:nt],
                                 AF.Relu, scale=1.0 / WSC)

        # mm2: r = W2.T @ h1 + (x + b2)   (W2 raw fp8, vector residual add)
        r_bf = rp.tile([128, KC, nt], BF16, tag="r")
        for g in range(KC // 2):
            ps = psum.tile([128, 2, 512], F32, tag="mm", bufs=3)
            for j in range(2):
                mc = 2 * g + j
                sl = slice(mc * 128, (mc + 1) * 128)
                for kk in range(0, KH, 2):
                    nc.tensor.matmul(ps[:, j, 0:nt], wtiles["W_mi2"][:, kk:kk + 2, sl],
                                     h1[:, kk:kk + 2, :], perf_mode=DR,
                                     start=(kk == 0), stop=(kk == KH - 2),
                                     skip_group_check=True)
            nc.vector.tensor_add(r_bf[:, 2 * g:2 * g + 2, :], ps[:, :, 0:nt],
                                 xb[:, 2 * g:2 * g + 2, :])
        return x8, xb, ind, r_bf

    def stage2(ti, s1):
        """attention + sc GEMM + f1 GEMM -> out tile store."""
        tl = tiles[ti]
        nt = _pad16(tl["nt"])
        nb, b0 = tl["nb"], tl["b0"]
        x8, xb, ind, r_bf = s1
        # qw_scr is written by qstage_b; stage2 is always emitted after it,
        # so this read is RAW-ordered correctly
        qw_loc = sp.tile([nb, D], BF16, tag="qwloc")
        nc.gpsimd.dma_start(qw_loc[:], qw_scr[b0:b0 + nb, :])

        # block-diag attention scores (bf16)
        at = psum.tile([nb, nt], F32, tag="at", bufs=1)
        for kc in range(KC):
            nc.tensor.matmul(at[:], qT_bf[:, kc, b0:b0 + nb], r_bf[:, kc, :],
                             start=(kc == 0), stop=(kc == KC - 1))
        masked = sp.tile([nb, nt], BF16, tag="msk")
        nc.vector.tensor_tensor(masked[:], at[:], ind[:], op=ALU.mult)
        # fused reduce-over-rows + broadcast-to-128-partitions
        wps = psum.tile([128, nt], F32, tag="w", bufs=1)
        nc.tensor.matmul(wps[:], ones_sq[0:nb, :], masked[:], start=True, stop=True)
        w_bc = sp.tile([128, nt], BF16, tag="wbc")
        nc.scalar.activation(w_bc[:], wps[:], AF.Sigmoid,
                             scale=float(1.0 / np.sqrt(D)))

        # wc = w * r  (fp8 for the sc GEMM)
        wc8 = wcp.tile([128, KC, nt], F8, tag="wc")
        for kc in range(KC):
            nc.vector.tensor_mul(wc8[:, kc, :], r_bf[:, kc, :], w_bc[:])

        # mm3: scaling = tanh((16*sc_W).T @ wc / 16  [+ sc_b])
        scal = scp.tile([128, KC, nt], BF16, tag="scal")
        for g in range(KC // 2):
            ps = psum.tile([128, 2, 512], F32, tag="mm", bufs=3)
            for j in range(2):
                mc = 2 * g + j
                sl = slice(mc * 128, (mc + 1) * 128)
                for kk in range(0, KC, 2):
                    nc.tensor.matmul(ps[:, j, 0:nt], wtiles["W_sc"][:, kk:kk + 2, sl],
                                     wc8[:, kk:kk + 2, :], perf_mode=DR,
                                     start=(kk == 0),
                                     stop=(kk == KC - 2 and not has_scb),
                                     skip_group_check=True)
                if has_scb:
                    nc.tensor.matmul(ps[:, j, 0:nt], wtiles["scb_row"][0:1, sl],
                                     ones_row[0:1, 0:nt],
                                     start=False, stop=True,
                                     skip_group_check=True)
            nc.scalar.activation(scal[:, 2 * g:2 * g + 2, :], ps[:, :, 0:nt],
                                 AF.Tanh, scale=1.0 / WSC)

        # z = wc * scaling (fp8); mm4: out = relu((16*f1_W).T @ z + 16*qw)/16
        z8 = zp.tile([128, KC, nt], F8, tag="z")
        for g in range(2):
            nc.vector.tensor_mul(z8[:, 4 * g:4 * g + 4, :],
                                 wc8[:, 4 * g:4 * g + 4, :],
                                 scal[:, 4 * g:4 * g + 4, :])
        o_bf = op.tile([128, KC, nt], BF16, tag="o")
        for g in range(KC // 2):
            ps = psum.tile([128, 2, 512], F32, tag="mm", bufs=3)
            for j in range(2):
                mc = 2 * g + j
                sl = slice(mc * 128, (mc + 1) * 128)
                for kk in range(0, KC, 2):
                    nc.tensor.matmul(ps[:, j, 0:nt], wtiles["W_f1"][:, kk:kk + 2, sl],
                                     z8[:, kk:kk + 2, :], perf_mode=DR,
                                     start=(kk == 0), stop=False,
                                     skip_group_check=True)
                nc.tensor.matmul(ps[:, j, 0:nt], qw_loc[:, sl], ind[:],
                                 start=False, stop=True, skip_group_check=True)
            nc.scalar.activation(o_bf[:, 2 * g:2 * g + 2, :], ps[:, :, 0:nt],
                                 AF.Relu, scale=1.0 / WSC)
        nc.sync.dma_start(
            outblob[ti, :, 0:KC * nt].rearrange("p (kc t) -> p kc t", kc=KC),
            o_bf[:])

    # ======================= q-stage (two parts) =========================
    qp = tc.tile_pool(name="qpool", bufs=1)
    qpc = qp.__enter__()

    def qstage_loads():
        """issue q-stage input DMAs early (Pool queue) so unet lands first."""
        unet_sb = qpc.tile([BC, SEG_C, 49], BF16)
        nc.gpsimd.dma_start(unet_sb[:], unet[:, :, :])
        W_seg_a = qpc.tile([128, D], BF16)
        nc.gpsimd.dma_start(W_seg_a[:], wi["seg_W"][0:128, :])
        W_seg_b = qpc.tile([5, D], BF16)
        nc.gpsimd.dma_start(W_seg_b[:], wi["seg_W"][128:SEG_C, :])
        b_segr = qpc.tile([1, D], BF16)
        nc.gpsimd.dma_start(b_segr[:], wi["seg_b"])
        g_bc = qpc.tile([128, D], BF16)
        nc.gpsimd.dma_start(g_bc[:], bass.AP(tensor=wi["ln_g"].tensor,
                                             offset=wi["ln_g"].offset,
                                             ap=[[0, 128], [1, D]]))
        bb_bc = qpc.tile([128, D], BF16)
        nc.gpsimd.dma_start(bb_bc[:], bass.AP(tensor=wi["ln_b"].tensor,
                                              offset=wi["ln_b"].offset,
                                              ap=[[0, 128], [1, D]]))
        return unet_sb, W_seg_a, W_seg_b, b_segr, g_bc, bb_bc

    def qstage_a(loads):
        """unet pooling + seg embedding + layernorm -> qn_bf (token-major)."""
        unet_sb, W_seg_a, W_seg_b, b_segr, g_bc, bb_bc = loads

        pooled = qpc.tile([BC, SEG_C], F32)
        nc.vector.reduce_sum(pooled[:], unet_sb[:], axis=mybir.AxisListType.X)
        pooled_bf = qpc.tile([BC, SEG_C], BF16)
        nc.scalar.mul(pooled_bf[:], pooled[:], 1.0 / 49.0)
        pa_ps = psum.tile([128, BC], BF16, tag="w", bufs=1)
        nc.tensor.transpose(pa_ps[:], pooled_bf[:, 0:128], ident_bf[0:BC, 0:BC])
        pa_bf = qpc.tile([128, BC], BF16)
        nc.scalar.copy(pa_bf[:], pa_ps[:])
        pb_ps = psum.tile([5, BC], BF16, tag="w", bufs=1)
        nc.tensor.transpose(pb_ps[:], pooled_bf[:, 128:SEG_C], ident_bf[0:BC, 0:BC])
        pb_bf = qpc.tile([5, BC], BF16)
        nc.scalar.copy(pb_bf[:], pb_ps[:])

        # q1 = relu(pooled @ seg_W + seg_b)   (token-major: BC x D)
        q1 = qpc.tile([BC, D], F32)
        for ng in range(2):
            sl = slice(ng * 512, (ng + 1) * 512)
            ps = psum.tile([BC, 512], F32, tag="mm", bufs=3)
            nc.tensor.matmul(ps[:], pa_bf[:], W_seg_a[:, sl], start=True, stop=False)
            nc.tensor.matmul(ps[:], pb_bf[:], W_seg_b[:, sl], start=False, stop=False)
            nc.tensor.matmul(ps[:], ones_row[0:1, 0:BC], b_segr[0:1, sl],
                             start=False, stop=True)
            nc.vector.tensor_scalar_max(q1[:, sl], ps[:], 0.0)

        # layernorm over D
        stats = qpc.tile([BC, 2, 6], F32)
        for s in range(2):
            nc.vector.bn_stats(stats[:, s, :], q1[:, s * 512:(s + 1) * 512])
        mv = qpc.tile([BC, 2], F32)
        nc.vector.bn_aggr(mv[:], stats[:])
        rstd = qpc.tile([BC, 1], F32)
        nc.scalar.activation(rstd[:], mv[:, 1:2], AF.Sqrt, bias=eps_t[0:BC, :])
        nc.vector.reciprocal(rstd[:], rstd[:])
        qn = qpc.tile([BC, D], F32)
        nc.vector.tensor_scalar(qn[:], q1[:], mv[:, 0:1], rstd[:],
                                op0=ALU.subtract, op1=ALU.mult)
        nc.vector.tensor_mul(qn[:], qn[:], g_bc[0:BC, :])
        qn_bf = qpc.tile([BC, D], BF16)
        nc.vector.tensor_add(qn_bf[:], qn[:], bb_bc[0:BC, :])
        return qn_bf

    def qstage_b(qn_bf):
        """q MLP (bf16) -> qT_bf; qw = 16*(q @ f1_W + f1_b) -> qw_scr."""
        W_ms1 = qpc.tile([128, KC, H], BF16)
        nc.gpsimd.dma_start(W_ms1[:], wi["ms_W1"].rearrange("p (kc m) -> p kc m", kc=KC))
        W_ms2 = qpc.tile([128, KH, D], BF16)
        nc.gpsimd.dma_start(W_ms2[:], wi["ms_W2"].rearrange("p (kc m) -> p kc m", kc=KH))
        b_ms1r = qpc.tile([1, H], BF16)
        nc.gpsimd.dma_start(b_ms1r[:], wi["ms_b1"])
        b_ms2r = qpc.tile([1, D], BF16)
        nc.gpsimd.dma_start(b_ms2r[:], wi["ms_b2"])
        fb_bc = qpc.tile([BC, D], BF16)
        nc.gpsimd.dma_start(fb_bc[:], bass.AP(tensor=wi["f1_b"].tensor,
                                              offset=wi["f1_b"].offset,
                                              ap=[[0, BC], [1, D]]))

        # qnT (feature-major) via PE transposes
        qnT_bf = qpc.tile([128, KC, BC], BF16)
        for kc in range(KC):
            pt = psum.tile([128, BC], BF16, tag="w", bufs=1)
            nc.tensor.transpose(pt[:], qn_bf[:, kc * 128:(kc + 1) * 128],
                                ident_bf[0:BC, 0:BC])
            nc.scalar.copy(qnT_bf[:, kc, :], pt[:])

        # q MLP (feature-major): qm = relu(ms_W1.T @ qnT + b1)
        qmT_bf = qpc.tile([128, KH, BC], BF16)
        for mc in range(KH):
            sl = slice(mc * 128, (mc + 1) * 128)
            ps = psum.tile([128, BC], F32, tag="mm", bufs=3)
            for kc in range(KC):
                nc.tensor.matmul(ps[:], W_ms1[:, kc, sl], qnT_bf[:, kc, :],
                                 start=(kc == 0), stop=False)
            nc.tensor.matmul(ps[:], b_ms1r[0:1, sl], ones_row[0:1, 0:BC],
                             start=False, stop=True)
            nc.scalar.activation(qmT_bf[:, mc, :], ps[:], AF.Relu)
        # q2T = ms_W2.T @ qmT + b2 + qnT   -> qT_bf
        for mc in range(KC):
            sl = slice(mc * 128, (mc + 1) * 128)
            ps = psum.tile([128, BC], F32, tag="mm", bufs=3)
            for kc in range(KH):
                nc.tensor.matmul(ps[:], W_ms2[:, kc, sl], qmT_bf[:, kc, :],
                                 start=(kc == 0), stop=False)
            nc.tensor.matmul(ps[:], b_ms2r[0:1, sl], ones_row[0:1, 0:BC],
                             start=False, stop=True)
            nc.vector.tensor_add(qT_bf[:, mc, :], ps[:], qnT_bf[:, mc, :])

        # qw = 16*(q2 @ f1_W + f1_b) (token-major); DRAM scratch for row slices
        qw_bf = qpc.tile([BC, D], BF16)
        for ng in range(2):
            sl = slice(ng * 512, (ng + 1) * 512)
            W_f1h = qpc.tile([128, KC, 512], BF16, tag="wf1h", bufs=1)
            nc.gpsimd.dma_start(
                W_f1h[:], wi["f1_Wb"][ng].rearrange("p (kc m) -> p kc m", kc=KC))
            ps = psum.tile([BC, 512], F32, tag="mm", bufs=3)
            for kc in range(KC):
                nc.tensor.matmul(ps[:], qT_bf[:, kc, :], W_f1h[:, kc, :],
                                 start=(kc == 0), stop=(kc == KC - 1))
            nc.vector.tensor_add(qw_bf[:, sl], ps[:], fb_bc[0:BC, sl])
        nc.scalar.mul(qw_bf[:], qw_bf[:], WSC)
        nc.gpsimd.dma_start(qw_scr[:, :], qw_bf[:])

    # ======================= emission order ==============================
    load_main_weights()
    qloads = qstage_loads()
    s1_results = {}
    s1_results[0] = stage1(0)
    if ntiles > 1:
        s1_results[1] = stage1(1)
    qn_bf = qstage_a(qloads)
    qstage_b(qn_bf)
    qp.__exit__(None, None, None)

    for ti in range(ntiles):
        stage2(ti, s1_results.pop(ti))
        nxt = ti + 2
        if nxt < ntiles:
            s1_results[nxt] = stage1(nxt)


def _build(plan, has_b1, has_scb):
    nc = bacc.Bacc("TRN2", target_bir_lowering=False, debug=False)
    ctx = ExitStack()
    with tile.TileContext(nc) as tc, ctx:
        _emit(ctx, tc, plan, has_b1, has_scb)
    nc.compile()
    return nc


_NC_CACHE = {}


def _get_nc(plan_key, plan, has_b1, has_scb):
    if plan_key not in _NC_CACHE:
        _NC_CACHE[plan_key] = _build(plan, has_b1, has_scb)
    return _NC_CACHE[plan_key]


def _build_ind_blob(tiles):
    sz = sum(t["nb"] * _pad16(t["nt"]) for t in tiles)
    blob = np.zeros(sz, dtype=NPBF16)
    off = 0
    for t in tiles:
        ntp = _pad16(t["nt"])
        ind = np.zeros((t["nb"], ntp), dtype=NPBF16)
        for row, lo, hi in t["segs"]:
            ind[row, lo:hi] = 1
        blob[off:off + ind.size] = ind.ravel()
        off += ind.size
    return blob


def _run_cores(ncs, in_maps, trace=False):
    """Dispatch one compiled program per core, concurrently."""
    import jax
    from concourse import bass2jax
    from concourse.bass2jax import _bass_exec_p, install_neuronx_cc_hook

    install_neuronx_cc_hook()
    devices = jax.devices()[:NCORES]

    def make_jit(nc):
        in_names, out_names, out_avals, zero_outs = [], [], [], []
        for alloc in nc.m.functions[0].allocations:
            if not isinstance(alloc, mybir.MemoryLocationSet):
                continue
            name = alloc.memorylocations[0].name
            if alloc.kind == "ExternalInput":
                in_names.append(name)
            elif alloc.kind == "ExternalOutput":
                out_names.append(name)
                shape = tuple(alloc.tensor_shape)
                dtype = mybir.dt.np(alloc.dtype)
                out_avals.append(jax.core.ShapedArray(shape, dtype))
                zero_outs.append(np.zeros(shape, dtype))
        n_params = len(in_names)
        all_names = in_names + out_names

        def _body(*args):
            outs = _bass_exec_p.bind(
                *args,
                out_avals=tuple(out_avals),
                in_names=tuple(all_names),
                out_names=tuple(out_names),
                lowering_input_output_aliases=(),
                sim_require_finite=True,
                sim_require_nnan=True,
                nc=nc,
            )
            return tuple(outs)

        donate = tuple(range(n_params, n_params + len(out_names)))
        return (jax.jit(_body, donate_argnums=donate, keep_unused=True),
                in_names, out_names, zero_outs)

    with ThreadPoolExecutor(NCORES) as ex:
        jits = list(ex.map(make_jit, ncs))

    def launch(c):
        jitted, in_names, out_names, zero_outs = jits[c]
        vals = dict(in_maps[c])
        pid = ncs[c].partition_id_tensor
        if pid is not None:
            vals[pid.name] = np.array([[c]], dtype=np.uint32)
        args = [jax.device_put(np.asarray(vals[n]), devices[c]) for n in in_names]
        zz = [jax.device_put(z, devices[c]) for z in zero_outs]
        outs = jitted(*args, *zz)
        return dict(zip(out_names, outs))

    def run_all():
        with ThreadPoolExecutor(NCORES) as ex:
            outs = list(ex.map(launch, range(NCORES)))
        return [{k: np.asarray(v) for k, v in o.items()} for o in outs]

    global LAST_EXEC_NS, _LAST_TRACE
    if trace:
        import glob as globmod
        import tempfile
        from antenv.axon_hooks import get_axon_ntff_profile_hook
        hook = get_axon_ntff_profile_hook()
        neff_dir = tempfile.mkdtemp()
        if hook is None:
            results = run_all()
        else:
            run_all()  # warm: jit trace + NEFF compile before the profiled run
            with hook(neff_dir, [0]):
                results = run_all()
            try:
                import re
                import shutil
                import gauge.profiler
                from concourse._compat import FishPath
                ntffs = sorted(globmod.glob(os.path.join(neff_dir, "*_body*.ntff")))
                times = []
                insts_best = None
                for ntff in ntffs:
                    m = re.search(r"executable(\d+)", os.path.basename(ntff))
                    exe = m.group(1)
                    sub = os.path.join(neff_dir, f"exe{exe}")
                    os.makedirs(sub, exist_ok=True)
                    for fpath in globmod.glob(os.path.join(neff_dir, f"*executable{exe}*")):
                        if os.path.isfile(fpath):
                            shutil.copy(fpath, sub)
                    profile = gauge.profiler.Profile(
                        profile_path=FishPath(sub), kernel_dev_mode=True,
                        profile_on_exit=False, bass_kernel=ncs[0].m,
                        offline_processing=True, fname="*_body*",
                        metadata={"artifacts_path": sub})
                    pr = profile.to_perfetto(model_index=(0,))
                    if pr:
                        times.append(pr[0].exec_time_ns)
                        if pr[0].exec_time_ns == max(times):
                            insts_best = (pr[0].insts, pr[0].trace_path)
                if times:
                    LAST_EXEC_NS = max(times)
                    _LAST_TRACE = insts_best
                    print(f"per-core exec ns: {sorted(times)}", file=sys.stderr)
                    print(f"neff_dir: {neff_dir}", file=sys.stderr)
            except Exception as e:
                print(f"profile post-processing failed: {e!r}", file=sys.stderr)
    else:
        results = run_all()
    return results


def kernel(rgns, Unet_segs, region_lens, mi_W1, mi_b1, mi_W2, mi_b2,
           ms_W1, ms_b1, ms_W2, ms_b2, seg_W, seg_b, ln_g, ln_b,
           sc_W, sc_b, f1_W, f1_b):
    _wire_ntff_hook()

    f = lambda a: np.ascontiguousarray(np.asarray(a, dtype=np.float32))
    bf = lambda a: np.ascontiguousarray(np.asarray(a, dtype=np.float32).astype(NPBF16))
    f8s = lambda a: np.ascontiguousarray(
        (np.asarray(a, dtype=np.float32) * WSC).astype(NPF8))
    f8r = lambda a: np.ascontiguousarray(np.asarray(a, dtype=np.float32).astype(NPF8))
    rgns = f(rgns)
    unet = bf(Unet_segs).reshape(B, SEG_C, 49)
    lens = np.clip(np.asarray(region_lens).astype(np.int64), 0, R)

    def pack(w, kchunks):
        w = np.asarray(w)
        m = w.shape[1]
        return np.ascontiguousarray(
            w.reshape(kchunks, 128, m).transpose(1, 0, 2).reshape(128, kchunks * m))

    mi_b1_ = f(mi_b1).reshape(1, H)
    sc_b_ = f(sc_b).reshape(1, D)
    has_b1 = bool(np.any(mi_b1_ != 0))
    has_scb = bool(np.any(sc_b_ != 0))

    f1_Wb_ = bf(f1_W)                      # (D, D) bf16
    f1_Wb_halves = np.stack([pack(f1_Wb_[:, 0:512], KC),
                             pack(f1_Wb_[:, 512:1024], KC)])
    weights = {
        "mi_W1": pack(f8s(mi_W1), KC), "mi_b1": (mi_b1_ * WSC).astype(NPBF16),
        "mi_W2": pack(f8r(mi_W2), KH),
        "ms_W1": pack(bf(ms_W1), KC), "ms_b1": bf(ms_b1).reshape(1, H),
        "ms_W2": pack(bf(ms_W2), KH), "ms_b2": bf(ms_b2).reshape(1, D),
        "seg_W": bf(seg_W), "seg_b": bf(seg_b).reshape(1, D),
        "ln_g": bf(ln_g).reshape(1, D), "ln_b": bf(ln_b).reshape(1, D),
        "sc_W": pack(f8s(sc_W), KC), "sc_b": (sc_b_ * WSC).astype(NPBF16),
        "f1_W": pack(f8s(f1_W), KC), "f1_Wb": f1_Wb_halves,
        "f1_b": bf(f1_b).reshape(1, D),
    }

    # balanced batch assignment: 128 batches per core, equalize token counts
    order = np.argsort(-lens, kind="stable")
    loads = np.zeros(NCORES, dtype=np.int64)
    counts = np.zeros(NCORES, dtype=np.int64)
    assign = [[] for _ in range(NCORES)]
    for b in order:
        open_cores = [c for c in range(NCORES) if counts[c] < BC]
        c = min(open_cores, key=lambda c: loads[c])
        assign[c].append(int(b))
        loads[c] += int(lens[b])
        counts[c] += 1
    batches = [np.sort(np.array(a, dtype=np.int64)) for a in assign]

    rflat = rgns.reshape(B * R, D)
    b2row = f(mi_b2).reshape(1, D)
    in_maps, plans, vrows, tileinfo = [], [], [], []
    for c in range(NCORES):
        bl = batches[c]
        lens_c = lens[bl]
        plan = _make_plan(lens_c)
        plans.append(plan)
        tiles = plan[0]
        ntiles = len(tiles)
        rows = np.concatenate([bl[i] * R + np.arange(lens_c[i]) for i in range(BC)])
        vrows.append(rows)
        xv = rflat[rows]                      # (ntokc, D) f32
        x8b = np.zeros((ntiles, 128, KC * TOKCAP), dtype=NPF8)
        xbb = np.zeros((ntiles, 128, KC * TOKCAP), dtype=NPBF16)
        xvb2 = (xv + b2row).astype(NPBF16)
        xv8 = xv.astype(NPF8)
        for ti, tl in enumerate(tiles):
            t0, nt = tl["t0"], tl["nt"]
            ntp = _pad16(nt)
            # (nt, D) -> (128, KC, ntp)
            blk8 = xv8[t0:t0 + nt].reshape(nt, KC, 128).transpose(2, 1, 0)
            blkb = xvb2[t0:t0 + nt].reshape(nt, KC, 128).transpose(2, 1, 0)
            x8b[ti, :, :KC * ntp].reshape(128, KC, ntp)[:, :, :nt] = blk8
            xbb[ti, :, :KC * ntp].reshape(128, KC, ntp)[:, :, :nt] = blkb
        tileinfo.append(tiles)
        in_maps.append(dict(
            x8blob=x8b,
            xbblob=xbb,
            unet=np.ascontiguousarray(unet[bl]),
            ind=_build_ind_blob(tiles),
            **weights,
        ))

    def plan_key(c):
        return tuple((t["t0"], t["nt"], t["b0"], t["nb"], tuple(t["segs"]))
                     for t in plans[c][0])

    keys = [plan_key(c) for c in range(NCORES)]
    uniq = {}
    for c in range(NCORES):
        if keys[c] not in uniq:
            uniq[keys[c]] = None
    with ThreadPoolExecutor(min(8, len(uniq))) as ex:
        built = dict(zip(uniq.keys(),
                         ex.map(lambda k: _get_nc(k, plans[keys.index(k)],
                                                  has_b1, has_scb),
                                list(uniq.keys()))))
    ncs = [built[keys[c]] for c in range(NCORES)]

    trace = bool(int(os.environ.get("BASSK_TRACE", "0")))
    results = _run_cores(ncs, in_maps, trace=trace)

    out = np.empty((B * R, D), np.float32)
    out[:] = np.maximum(f(f1_b).reshape(1, D), 0.0)
    for c in range(NCORES):
        ob = results[c]["outblob"]            # (ntiles, 128, KC*TOKCAP) bf16
        tiles = tileinfo[c]
        ntokc = plans[c][1]
        res = np.empty((ntokc, D), np.float32)
        for ti, tl in enumerate(tiles):
            t0, nt = tl["t0"], tl["nt"]
            ntp = _pad16(nt)
            blk = ob[ti, :, :KC * ntp].reshape(128, KC, ntp)[:, :, :nt]
            res[t0:t0 + nt] = blk.transpose(2, 1, 0).reshape(nt, D).astype(np.float32)
        out[vrows[c]] = res
    return out.reshape(B, R, D)
